# revision 11
# baseline (speedup 1.0000x reference)
"""12-bit ripple-carry adder (SNN gate semantics) on 8 TRN2 NeuronCores.

Inputs A, B: (4194304, 12) float32 binary {0,1}, bit 11 = LSB.
Returns (sum_bits (4194304, 12) f32, carry_out (4194304, 1) f32), bit-exact
vs the reference (all values are exact small integers in f32).

Data parallel, no collectives; radix-4 carry chain. Per tile (128 partitions
x F rows, 12 bits contiguous per row, 6 bit-pairs per row):
  T  = A + B                 on GPSIMD            (12F elems)
  PV = 2*T_even + T_odd      STT on DVE           (6F, pair values 0..6)
  VAL_k = PV_k + c_in        ONE prefix scan over pairs in reversed order
                             (LSB pair first):  state' = (G4 is_le state) + PV
                             G4 = 4.0, 10.0 at each row's LSB pair (resets
                             the carry at row boundaries). VAL in 0..7.
  sg4 = Sign(VAL - 3.5)      ACT   (= 2*c_out-1 per pair)
  R2  = VAL - 2*sg4          STT   (= (VAL mod 4) + 2, in 2..5)
  sg2 = Sign(R2 - 3.5)       ACT   (= 2*s_high-1)
  s_low  = R2 - sg2 - 3      STT -> odd output columns
  s_high = (sg2 + 1)/2       ACT copy -> even output columns
  carry  = (sg4|pair0 + 1)/2 ACT copy
All DMAs are plain HWDGE at line rate (loads on sync, stores on scalar).
"""
import numpy as np
import concourse.tile as tile
from concourse import bacc, mybir
from concourse.bass_utils import run_bass_kernel_spmd

N_BITS = 12
NPAIR = N_BITS // 2
BATCH = 4_194_304
N_CORES = 8
SHARD = BATCH // N_CORES        # 524288 rows per core
P = 128                         # SBUF partitions
F = 128                         # rows per partition per tile
ROWS_PER_TILE = P * F           # 32768
TILES = SHARD // ROWS_PER_TILE  # 32
W = N_BITS * F                  # 12F: full-width free elems per partition
WP = NPAIR * F                  # 6F: pair-domain free elems per partition


def _build():
    nc = bacc.Bacc("TRN2", target_bir_lowering=False, debug=False,
                   num_devices=N_CORES)
    A = nc.dram_tensor("A", [SHARD, N_BITS], mybir.dt.float32, kind="ExternalInput")
    B = nc.dram_tensor("B", [SHARD, N_BITS], mybir.dt.float32, kind="ExternalInput")
    S = nc.dram_tensor("S", [SHARD, N_BITS], mybir.dt.float32, kind="ExternalOutput")
    C = nc.dram_tensor("C", [SHARD, 1], mybir.dt.float32, kind="ExternalOutput")

    with tile.TileContext(nc) as tc:
        with tc.tile_pool(name="const", bufs=1) as constp, \
             tc.tile_pool(name="work", bufs=6) as work:
            # Pair-scan gate/threshold: 4.0 everywhere, 10.0 at pair 5 (LSB).
            G4 = constp.tile([P, WP], mybir.dt.float32)
            nc.vector.memset(G4[:], 4.0)
            G4v = G4[:].rearrange("p (f k) -> p f k", k=NPAIR)
            nc.vector.memset(G4v[:, :, NPAIR - 1 : NPAIR], 10.0)
            bneg = constp.tile([P, 1], mybir.dt.float32)
            nc.vector.memset(bneg[:], -3.5)

            for t in range(TILES):
                r0, r1 = t * ROWS_PER_TILE, (t + 1) * ROWS_PER_TILE
                Av = A[r0:r1, :].rearrange("(p f) b -> p (f b)", p=P)
                Bv = B[r0:r1, :].rearrange("(p f) b -> p (f b)", p=P)
                Sv = S[r0:r1, :].rearrange("(p f) b -> p (f b)", p=P)
                Cv = C[r0:r1, :].rearrange("(p f) b -> p (f b)", p=P)

                XA = work.tile([P, W], mybir.dt.float32)   # A -> T
                XB = work.tile([P, W], mybir.dt.float32)   # B -> S
                PV = work.tile([P, WP], mybir.dt.float32)  # pair values -> R2
                VA = work.tile([P, WP], mybir.dt.float32)  # scan out VAL
                s4 = work.tile([P, WP], mybir.dt.float32)  # sign(VAL-3.5)
                s2 = work.tile([P, WP], mybir.dt.float32)  # sign(R2-3.5)
                co = work.tile([P, F], mybir.dt.float32)   # final carry per row

                nc.sync.dma_start(out=XA[:], in_=Av)
                nc.sync.dma_start(out=XB[:], in_=Bv)

                nc.gpsimd.tensor_tensor(out=XA[:], in0=XA[:], in1=XB[:],
                                        op=mybir.AluOpType.add)

                T3 = XA[:].rearrange("p (f b) -> p f b", b=N_BITS)
                PV3 = PV[:].rearrange("p (f k) -> p f k", k=NPAIR)
                # PV = 2*T_even + T_odd
                nc.vector.scalar_tensor_tensor(
                    out=PV3[:, :, :], in0=T3[:, :, 0::2], scalar=2.0,
                    in1=T3[:, :, 1::2],
                    op0=mybir.AluOpType.mult, op1=mybir.AluOpType.add,
                )

                # radix-4 carry scan over pairs, reversed (LSB pair first).
                # WP = 1536 <= 2040, so a single instruction per tile.
                nc.vector.tensor_tensor_scan(
                    out=VA[:][:, ::-1], data0=G4[:][:, ::-1],
                    data1=PV[:][:, ::-1], initial=0.0,
                    op0=mybir.AluOpType.is_le, op1=mybir.AluOpType.add,
                )

                # sg4 = sign(VAL - 3.5)
                nc.scalar.activation(s4[:], VA[:],
                                     mybir.ActivationFunctionType.Sign,
                                     bias=bneg[:], scale=1.0)
                # R2 = VAL - 2*sg4  (= (VAL mod 4) + 2), into PV (dead)
                nc.vector.scalar_tensor_tensor(
                    out=PV[:], in0=s4[:], scalar=-2.0, in1=VA[:],
                    op0=mybir.AluOpType.mult, op1=mybir.AluOpType.add,
                )
                # sg2 = sign(R2 - 3.5)
                nc.scalar.activation(s2[:], PV[:],
                                     mybir.ActivationFunctionType.Sign,
                                     bias=bneg[:], scale=1.0)

                S3 = XB[:].rearrange("p (f b) -> p f b", b=N_BITS)
                # s_low = (R2 - 3) - sg2 -> odd columns
                nc.vector.scalar_tensor_tensor(
                    out=S3[:, :, 1::2], in0=PV3[:, :, :], scalar=-3.0,
                    in1=s2[:].rearrange("p (f k) -> p f k", k=NPAIR),
                    op0=mybir.AluOpType.add, op1=mybir.AluOpType.subtract,
                )
                # s_high = (sg2 + 1)/2 -> even columns
                nc.scalar.activation(S3[:, :, 0::2],
                                     s2[:].rearrange("p (f k) -> p f k", k=NPAIR),
                                     mybir.ActivationFunctionType.Copy,
                                     bias=0.5, scale=0.5)
                # carry = (sg4|pair0 + 1)/2
                s4v = s4[:].rearrange("p (f k) -> p f k", k=NPAIR)
                nc.scalar.activation(co[:], s4v[:, :, 0:1],
                                     mybir.ActivationFunctionType.Copy,
                                     bias=0.5, scale=0.5)

                nc.scalar.dma_start(out=Sv, in_=XB[:])
                nc.scalar.dma_start(out=Cv, in_=co[:])
    nc.compile()
    return nc


_NC = None


def kernel(A: np.ndarray, B: np.ndarray):
    global _NC
    if _NC is None:
        _NC = _build()
    A = np.ascontiguousarray(A, dtype=np.float32)
    B = np.ascontiguousarray(B, dtype=np.float32)
    in_maps = [
        {"A": A[i * SHARD : (i + 1) * SHARD], "B": B[i * SHARD : (i + 1) * SHARD]}
        for i in range(N_CORES)
    ]
    res = run_bass_kernel_spmd(_NC, in_maps, core_ids=list(range(N_CORES)))
    S = np.concatenate([r["S"] for r in res.results], axis=0)
    C = np.concatenate([r["C"] for r in res.results], axis=0)
    return S, C


# revision 12
# speedup vs baseline: 1.0217x; 1.0217x over previous
"""12-bit ripple-carry adder (SNN gate semantics) on 8 TRN2 NeuronCores.

Inputs A, B: (4194304, 12) float32 binary {0,1}, bit 11 = LSB.
Returns (sum_bits (4194304, 12) f32, carry_out (4194304, 1) f32), bit-exact
vs the reference (all values are exact small integers in f32).

Data parallel, no collectives; radix-4 carry chain. Per tile (128 partitions
x F rows, 12 bits contiguous per row, 6 bit-pairs per row):
  T  = A + B                 on GPSIMD            (12F elems)
  PV = 2*T_even + T_odd      STT on DVE           (6F, pair values 0..6)
  VAL_k = PV_k + c_in        ONE prefix scan over pairs in reversed order
                             (LSB pair first):  state' = (G4 is_le state) + PV
                             G4 = 4.0, 10.0 at each row's LSB pair (resets
                             the carry at row boundaries). VAL in 0..7.
  sg4 = Sign(VAL - 3.5)      ACT   (= 2*c_out-1 per pair)
  R2  = VAL - 2*sg4          STT   (= (VAL mod 4) + 2, in 2..5)
  sg2 = Sign(R2 - 3.5)       ACT   (= 2*s_high-1)
  s_low  = R2 - sg2 - 3      STT -> odd output columns
  s_high = (sg2 + 1)/2       ACT copy -> even output columns
  carry  = (sg4|pair0 + 1)/2 ACT copy
All DMAs are plain HWDGE at line rate (loads on sync, stores on scalar).
Tiles are single-purpose and short-lived so pool slots recycle early and
loads of tile k+bufs never wait on stores of tile k.
"""
import numpy as np
import concourse.tile as tile
from concourse import bacc, mybir
from concourse.bass_utils import run_bass_kernel_spmd

N_BITS = 12
NPAIR = N_BITS // 2
BATCH = 4_194_304
N_CORES = 8
SHARD = BATCH // N_CORES        # 524288 rows per core
P = 128                         # SBUF partitions
F = 256                         # rows per partition per tile
ROWS_PER_TILE = P * F           # 32768
TILES = SHARD // ROWS_PER_TILE  # 16
W = N_BITS * F                  # 12F: full-width free elems per partition
WP = NPAIR * F                  # 6F: pair-domain free elems per partition


def _build():
    nc = bacc.Bacc("TRN2", target_bir_lowering=False, debug=False,
                   num_devices=N_CORES)
    A = nc.dram_tensor("A", [SHARD, N_BITS], mybir.dt.float32, kind="ExternalInput")
    B = nc.dram_tensor("B", [SHARD, N_BITS], mybir.dt.float32, kind="ExternalInput")
    S = nc.dram_tensor("S", [SHARD, N_BITS], mybir.dt.float32, kind="ExternalOutput")
    C = nc.dram_tensor("C", [SHARD, 1], mybir.dt.float32, kind="ExternalOutput")

    with tile.TileContext(nc) as tc:
        with tc.tile_pool(name="const", bufs=1) as constp, \
             tc.tile_pool(name="io", bufs=3) as iop, \
             tc.tile_pool(name="sout", bufs=3) as sp, \
             tc.tile_pool(name="pair", bufs=3) as pp:
            # Pair-scan gate/threshold: 4.0 everywhere, 10.0 at pair 5 (LSB).
            G4 = constp.tile([P, WP], mybir.dt.float32)
            nc.vector.memset(G4[:], 4.0)
            G4v = G4[:].rearrange("p (f k) -> p f k", k=NPAIR)
            nc.vector.memset(G4v[:, :, NPAIR - 1 : NPAIR], 10.0)
            bneg = constp.tile([P, 1], mybir.dt.float32)
            nc.vector.memset(bneg[:], -3.5)

            for t in range(TILES):
                r0, r1 = t * ROWS_PER_TILE, (t + 1) * ROWS_PER_TILE
                Av = A[r0:r1, :].rearrange("(p f) b -> p (f b)", p=P)
                Bv = B[r0:r1, :].rearrange("(p f) b -> p (f b)", p=P)
                Sv = S[r0:r1, :].rearrange("(p f) b -> p (f b)", p=P)
                Cv = C[r0:r1, :].rearrange("(p f) b -> p (f b)", p=P)

                XA = iop.tile([P, W], mybir.dt.float32)   # A -> T (dies at PV)
                XB = iop.tile([P, W], mybir.dt.float32)   # B (dies at T-add)
                St = sp.tile([P, W], mybir.dt.float32)    # sum bits out
                PV = pp.tile([P, WP], mybir.dt.float32)   # pair values -> R2
                VA = pp.tile([P, WP], mybir.dt.float32)   # VAL, then sg2
                s4 = pp.tile([P, WP], mybir.dt.float32)   # sign(VAL-3.5)
                co = pp.tile([P, F], mybir.dt.float32)    # final carry per row

                nc.sync.dma_start(out=XA[:], in_=Av)
                nc.sync.dma_start(out=XB[:], in_=Bv)

                nc.gpsimd.tensor_tensor(out=XA[:], in0=XA[:], in1=XB[:],
                                        op=mybir.AluOpType.add)

                T3 = XA[:].rearrange("p (f b) -> p f b", b=N_BITS)
                PV3 = PV[:].rearrange("p (f k) -> p f k", k=NPAIR)
                # PV = 2*T_even + T_odd
                nc.vector.scalar_tensor_tensor(
                    out=PV3[:, :, :], in0=T3[:, :, 0::2], scalar=2.0,
                    in1=T3[:, :, 1::2],
                    op0=mybir.AluOpType.mult, op1=mybir.AluOpType.add,
                )

                # radix-4 carry scan over pairs, reversed (LSB pair first).
                # WP = 1536 <= 2040, so a single instruction per tile.
                nc.vector.tensor_tensor_scan(
                    out=VA[:][:, ::-1], data0=G4[:][:, ::-1],
                    data1=PV[:][:, ::-1], initial=0.0,
                    op0=mybir.AluOpType.is_le, op1=mybir.AluOpType.add,
                )

                # sg4 = sign(VAL - 3.5)
                nc.scalar.activation(s4[:], VA[:],
                                     mybir.ActivationFunctionType.Sign,
                                     bias=bneg[:], scale=1.0)
                # carry = (sg4|pair0 + 1)/2
                s4v = s4[:].rearrange("p (f k) -> p f k", k=NPAIR)
                nc.scalar.activation(co[:], s4v[:, :, 0:1],
                                     mybir.ActivationFunctionType.Copy,
                                     bias=0.5, scale=0.5)
                # R2 = VAL - 2*sg4  (= (VAL mod 4) + 2), into PV (dead)
                nc.vector.scalar_tensor_tensor(
                    out=PV[:], in0=s4[:], scalar=-2.0, in1=VA[:],
                    op0=mybir.AluOpType.mult, op1=mybir.AluOpType.add,
                )
                # sg2 = sign(R2 - 3.5), into VA (dead)
                nc.scalar.activation(VA[:], PV[:],
                                     mybir.ActivationFunctionType.Sign,
                                     bias=bneg[:], scale=1.0)

                S3 = St[:].rearrange("p (f b) -> p f b", b=N_BITS)
                VA3 = VA[:].rearrange("p (f k) -> p f k", k=NPAIR)
                # s_low = (R2 - 3) - sg2 -> odd columns
                nc.vector.scalar_tensor_tensor(
                    out=S3[:, :, 1::2], in0=PV3[:, :, :], scalar=-3.0,
                    in1=VA3[:, :, :],
                    op0=mybir.AluOpType.add, op1=mybir.AluOpType.subtract,
                )
                # s_high = (sg2 + 1)/2 -> even columns
                nc.scalar.activation(S3[:, :, 0::2], VA3[:, :, :],
                                     mybir.ActivationFunctionType.Copy,
                                     bias=0.5, scale=0.5)

                nc.scalar.dma_start(out=Sv, in_=St[:])
                nc.scalar.dma_start(out=Cv, in_=co[:])
    nc.compile()
    return nc


_NC = None


def kernel(A: np.ndarray, B: np.ndarray):
    global _NC
    if _NC is None:
        _NC = _build()
    A = np.ascontiguousarray(A, dtype=np.float32)
    B = np.ascontiguousarray(B, dtype=np.float32)
    in_maps = [
        {"A": A[i * SHARD : (i + 1) * SHARD], "B": B[i * SHARD : (i + 1) * SHARD]}
        for i in range(N_CORES)
    ]
    res = run_bass_kernel_spmd(_NC, in_maps, core_ids=list(range(N_CORES)))
    S = np.concatenate([r["S"] for r in res.results], axis=0)
    C = np.concatenate([r["C"] for r in res.results], axis=0)
    return S, C


# revision 13
# speedup vs baseline: 1.5451x; 1.5123x over previous
"""12-bit ripple-carry adder (SNN gate semantics) on 8 TRN2 NeuronCores.

Inputs A, B: (4194304, 12) float32 binary {0,1}, bit 11 = LSB.
Returns (sum_bits (4194304, 12) f32, carry_out (4194304, 1) f32), bit-exact
vs the reference (all values are exact small integers in f32).

Data parallel, no collectives; radix-4 carry chain, all elementwise math on
the Vector engine (GPSIMD stays idle — it shares an SBUF port with DVE and
concurrent use slows both). Per tile (128 partitions x F rows, 12 bits
contiguous per row, 6 bit-pairs per row):
  PA = 2*A_even + A_odd      STT   (6F elems)
  PB = 2*B_even + B_odd      STT   (6F)
  PV = PA + PB               TT    (6F, pair values 0..6; note T=A+B is
                                    never needed on its own)
  VAL_k = PV_k + c_in        ONE prefix scan over pairs in reversed order
                             (LSB pair first):  state' = (G4 is_le state) + PV
                             G4 = 4.0, 10.0 at each row's LSB pair (resets
                             the carry at row boundaries). VAL in 0..7.
  sg4 = Sign(VAL - 3.5)      ACT   (= 2*c_out-1 per pair)
  R2  = VAL - 2*sg4          STT   (= (VAL mod 4) + 2, in 2..5)
  sg2 = Sign(R2 - 3.5)       ACT   (= 2*s_high-1)
  s_low  = R2 - sg2 - 3      STT -> odd output columns
  s_high = (sg2 + 1)/2       ACT copy -> even output columns
  carry  = (sg4|pair0 + 1)/2 ACT copy
All DMAs are plain HWDGE at line rate (loads on sync, stores on scalar).
"""
import numpy as np
import concourse.tile as tile
from concourse import bacc, mybir
from concourse.bass_utils import run_bass_kernel_spmd

N_BITS = 12
NPAIR = N_BITS // 2
BATCH = 4_194_304
N_CORES = 8
SHARD = BATCH // N_CORES        # 524288 rows per core
P = 128                         # SBUF partitions
F = 256                         # rows per partition per tile
ROWS_PER_TILE = P * F           # 32768
TILES = SHARD // ROWS_PER_TILE  # 16
W = N_BITS * F                  # 12F: full-width free elems per partition
WP = NPAIR * F                  # 6F: pair-domain free elems per partition


def _build():
    nc = bacc.Bacc("TRN2", target_bir_lowering=False, debug=False,
                   num_devices=N_CORES)
    A = nc.dram_tensor("A", [SHARD, N_BITS], mybir.dt.float32, kind="ExternalInput")
    B = nc.dram_tensor("B", [SHARD, N_BITS], mybir.dt.float32, kind="ExternalInput")
    S = nc.dram_tensor("S", [SHARD, N_BITS], mybir.dt.float32, kind="ExternalOutput")
    C = nc.dram_tensor("C", [SHARD, 1], mybir.dt.float32, kind="ExternalOutput")

    with tile.TileContext(nc) as tc:
        with tc.tile_pool(name="const", bufs=1) as constp, \
             tc.tile_pool(name="io", bufs=3) as iop, \
             tc.tile_pool(name="sout", bufs=3) as sp, \
             tc.tile_pool(name="pair", bufs=3) as pp:
            # Pair-scan gate/threshold: 4.0 everywhere, 10.0 at pair 5 (LSB).
            G4 = constp.tile([P, WP], mybir.dt.float32)
            nc.vector.memset(G4[:], 4.0)
            G4v = G4[:].rearrange("p (f k) -> p f k", k=NPAIR)
            nc.vector.memset(G4v[:, :, NPAIR - 1 : NPAIR], 10.0)
            bneg = constp.tile([P, 1], mybir.dt.float32)
            nc.vector.memset(bneg[:], -3.5)

            for t in range(TILES):
                r0, r1 = t * ROWS_PER_TILE, (t + 1) * ROWS_PER_TILE
                Av = A[r0:r1, :].rearrange("(p f) b -> p (f b)", p=P)
                Bv = B[r0:r1, :].rearrange("(p f) b -> p (f b)", p=P)
                Sv = S[r0:r1, :].rearrange("(p f) b -> p (f b)", p=P)
                Cv = C[r0:r1, :].rearrange("(p f) b -> p (f b)", p=P)

                XA = iop.tile([P, W], mybir.dt.float32)   # A (dies at PA)
                XB = iop.tile([P, W], mybir.dt.float32)   # B (dies at PB)
                St = sp.tile([P, W], mybir.dt.float32)    # sum bits out
                PA = pp.tile([P, WP], mybir.dt.float32)   # 2Ae+Ao -> PV -> R2
                PB = pp.tile([P, WP], mybir.dt.float32)   # 2Be+Bo
                VA = pp.tile([P, WP], mybir.dt.float32)   # VAL, then sg2
                s4 = pp.tile([P, WP], mybir.dt.float32)   # sign(VAL-3.5)
                co = pp.tile([P, F], mybir.dt.float32)    # final carry per row

                nc.sync.dma_start(out=XA[:], in_=Av)
                nc.sync.dma_start(out=XB[:], in_=Bv)

                A3 = XA[:].rearrange("p (f b) -> p f b", b=N_BITS)
                B3 = XB[:].rearrange("p (f b) -> p f b", b=N_BITS)
                PA3 = PA[:].rearrange("p (f k) -> p f k", k=NPAIR)
                PB3 = PB[:].rearrange("p (f k) -> p f k", k=NPAIR)
                nc.vector.scalar_tensor_tensor(
                    out=PA3[:, :, :], in0=A3[:, :, 0::2], scalar=2.0,
                    in1=A3[:, :, 1::2],
                    op0=mybir.AluOpType.mult, op1=mybir.AluOpType.add,
                )
                nc.vector.scalar_tensor_tensor(
                    out=PB3[:, :, :], in0=B3[:, :, 0::2], scalar=2.0,
                    in1=B3[:, :, 1::2],
                    op0=mybir.AluOpType.mult, op1=mybir.AluOpType.add,
                )
                # PV = PA + PB, into PA
                nc.vector.tensor_tensor(out=PA[:], in0=PA[:], in1=PB[:],
                                        op=mybir.AluOpType.add)

                # radix-4 carry scan over pairs, reversed (LSB pair first).
                # WP = 1536 <= 2040, so a single instruction per tile.
                nc.vector.tensor_tensor_scan(
                    out=VA[:][:, ::-1], data0=G4[:][:, ::-1],
                    data1=PA[:][:, ::-1], initial=0.0,
                    op0=mybir.AluOpType.is_le, op1=mybir.AluOpType.add,
                )

                # sg4 = sign(VAL - 3.5)
                nc.scalar.activation(s4[:], VA[:],
                                     mybir.ActivationFunctionType.Sign,
                                     bias=bneg[:], scale=1.0)
                # carry = (sg4|pair0 + 1)/2
                s4v = s4[:].rearrange("p (f k) -> p f k", k=NPAIR)
                nc.scalar.activation(co[:], s4v[:, :, 0:1],
                                     mybir.ActivationFunctionType.Copy,
                                     bias=0.5, scale=0.5)
                # R2 = VAL - 2*sg4  (= (VAL mod 4) + 2), into PA (dead)
                nc.vector.scalar_tensor_tensor(
                    out=PA[:], in0=s4[:], scalar=-2.0, in1=VA[:],
                    op0=mybir.AluOpType.mult, op1=mybir.AluOpType.add,
                )
                # sg2 = sign(R2 - 3.5), into VA (dead)
                nc.scalar.activation(VA[:], PA[:],
                                     mybir.ActivationFunctionType.Sign,
                                     bias=bneg[:], scale=1.0)

                S3 = St[:].rearrange("p (f b) -> p f b", b=N_BITS)
                VA3 = VA[:].rearrange("p (f k) -> p f k", k=NPAIR)
                # s_low = (R2 - 3) - sg2 -> odd columns
                nc.vector.scalar_tensor_tensor(
                    out=S3[:, :, 1::2], in0=PA3[:, :, :], scalar=-3.0,
                    in1=VA3[:, :, :],
                    op0=mybir.AluOpType.add, op1=mybir.AluOpType.subtract,
                )
                # s_high = (sg2 + 1)/2 -> even columns
                nc.scalar.activation(S3[:, :, 0::2], VA3[:, :, :],
                                     mybir.ActivationFunctionType.Copy,
                                     bias=0.5, scale=0.5)

                nc.scalar.dma_start(out=Sv, in_=St[:])
                nc.scalar.dma_start(out=Cv, in_=co[:])
    nc.compile()
    return nc


_NC = None


def kernel(A: np.ndarray, B: np.ndarray):
    global _NC
    if _NC is None:
        _NC = _build()
    A = np.ascontiguousarray(A, dtype=np.float32)
    B = np.ascontiguousarray(B, dtype=np.float32)
    in_maps = [
        {"A": A[i * SHARD : (i + 1) * SHARD], "B": B[i * SHARD : (i + 1) * SHARD]}
        for i in range(N_CORES)
    ]
    res = run_bass_kernel_spmd(_NC, in_maps, core_ids=list(range(N_CORES)))
    S = np.concatenate([r["S"] for r in res.results], axis=0)
    C = np.concatenate([r["C"] for r in res.results], axis=0)
    return S, C
